# revision 8
# baseline (speedup 1.0000x reference)
"""Trainium2 Bass kernel for Bahdanau (MLP) additive attention.

Reference computation (B=4, T=128, S=512, H=512):
    wq = dec @ Wq.T + bq                    [B,T,H]
    uh = enc @ Wc.T                         [B,S,H]
    scores[b,t,s] = sum_h v[h] * tanh(wq[b,t,h] + uh[b,s,h])
    align = softmax(scores, axis=-1)        [B,T,S]
    c = align @ enc                         [B,T,H]
    attn_h = [c, dec] @ Wo.T + bo           [B,T,H]
    returns (attn_h, align.transpose(1,0,2))

Sharding: pure data parallel over (batch, T-half) -> 8 cores, 64 queries
per core, no cross-core communication.

Per-core dataflow (all layouts keep the hidden index on partitions):
    PE:  wqT = Wq.T-chunks @ decT   (+bq via DVE)         [h, t]
    PE:  uhT = Wc.T-chunks @ encT                          [h, s]
    DVE: sum(t,hc) = uhT[hc] + wqT[hc, t]  (per-partition scalar add)
    ACT: tanh over big batched tiles [128, 4096]
    PE:  scores[0:64, :] += V_window(t, hc).T @ tanh(t,hc)  (padded-v
         stationary trick: column t of the 64-wide window holds v_chunk,
         all other columns zero, so one M=64 matmul accumulates row t and
         adds zero elsewhere)
    DVE/ACT: softmax rows (reduce_max, exp with -max bias + accum row sum,
         reciprocal, scale)
    PE:  transpose align -> alignT; cT = enc-chunks @ alignT;
         attn_hT = Wo.T-chunks @ [cT; decT]  (+bo via DVE)
"""

import numpy as np

B, T, S, H = 4, 128, 512, 512
P = 128
NH = H // P          # 4 h-chunks
NS = S // P          # 4 s-chunks
TC = 64              # queries per core
TB = 2               # queries per ACT batch
FD = TB * NH * S     # 4096 free dim of the batched tanh tile
N_CORES = 8

# "f32r" (full-rate fp32-reduced matmul), "bf16", or "f32" (4x slower PE)
SCORES_MODE = "f32r"

_cached = None


def _build():
    import concourse.bacc as bacc
    import concourse.tile as tile
    import concourse.mybir as mybir
    from concourse.masks import make_identity

    f32 = mybir.dt.float32
    AF = mybir.ActivationFunctionType

    if SCORES_MODE == "bf16":
        tanh_dt = mybir.dt.bfloat16
    elif SCORES_MODE == "f32r":
        tanh_dt = mybir.dt.float32r
    else:
        tanh_dt = f32

    nc = bacc.Bacc("TRN2", target_bir_lowering=False, debug=False,
                   num_devices=N_CORES)

    d_decT = nc.dram_tensor("decT", [P, NH * TC], f32, kind="ExternalInput")
    bf16 = mybir.dt.bfloat16
    d_enc = nc.dram_tensor("enc", [P, NS * H], bf16, kind="ExternalInput")
    d_decTb = nc.dram_tensor("decTb", [P, NH * TC], bf16, kind="ExternalInput")
    uh_in_dt = mybir.dt.float32r if SCORES_MODE == "f32r" else f32
    d_encT = nc.dram_tensor("encT", [P, NH * S], uh_in_dt, kind="ExternalInput")
    d_wqt = nc.dram_tensor("wqt", [P, NH * H], f32, kind="ExternalInput")
    d_wct = nc.dram_tensor("wct", [P, NH * H], uh_in_dt, kind="ExternalInput")
    d_wot = nc.dram_tensor("wot", [P, 2 * NH * H], bf16, kind="ExternalInput")
    d_V_dt = mybir.dt.bfloat16 if SCORES_MODE == "bf16" else f32
    d_V = nc.dram_tensor("V", [P, NH * 64], d_V_dt, kind="ExternalInput")
    d_bqb = nc.dram_tensor("bqb", [P, NH], f32, kind="ExternalInput")
    d_bob = nc.dram_tensor("bob", [P, NH], f32, kind="ExternalInput")

    d_align = nc.dram_tensor("align_out", [TC, S], f32, kind="ExternalOutput")
    d_attn = nc.dram_tensor("attn_out", [P, NH * TC], f32, kind="ExternalOutput")


    with tile.TileContext(nc) as tc:
        with (
            tc.tile_pool(name="consts", bufs=1) as consts,
            tc.tile_pool(name="sums", bufs=4) as sums,
            tc.tile_pool(name="tanhs", bufs=3) as tanhs,
            tc.tile_pool(name="small", bufs=4) as small,
            tc.tile_pool(name="psc", bufs=1, space="PSUM") as psum_sc,
            tc.tile_pool(name="pbig", bufs=2, space="PSUM") as psum_big,
            tc.tile_pool(name="psm", bufs=2, space="PSUM") as psum_sm,
        ):
            uh_dt = mybir.dt.float32r if SCORES_MODE == "f32r" else f32
            sb_decT = consts.tile([P, NH * TC], f32)
            nc.sync.dma_start(sb_decT[:], d_decT[:])
            sb_wqt = consts.tile([P, NH * H], f32)
            nc.sync.dma_start(sb_wqt[:], d_wqt[:])
            sb_wct = consts.tile([P, NH * H], uh_dt)
            nc.sync.dma_start(sb_wct[:], d_wct[:])
            sb_encT = consts.tile([P, NH * S], uh_dt)
            nc.sync.dma_start(sb_encT[:], d_encT[:])
            sb_V0 = consts.tile([P, NH * 64], d_V_dt)
            nc.sync.dma_start(sb_V0[:], d_V[:])
            sb_bqb = consts.tile([P, NH], f32)
            nc.sync.dma_start(sb_bqb[:], d_bqb[:])
            sb_bob = consts.tile([P, NH], f32)
            nc.sync.dma_start(sb_bob[:], d_bob[:])
            sb_enc = consts.tile([P, NS * H], bf16)
            nc.sync.dma_start(sb_enc[:], d_enc[:])
            sb_decTb = consts.tile([P, NH * TC], bf16)
            nc.sync.dma_start(sb_decTb[:], d_decTb[:])
            sb_wot = consts.tile([P, 2 * NH * H], bf16)
            nc.sync.dma_start(sb_wot[:], d_wot[:])
            ident = consts.tile([32, 32], f32)
            make_identity(nc, ident[:, :])

            # wqT[k, t] (+bq), chunked over k: sb_wqb[:, kc*TC + t]
            sb_wqb = consts.tile([P, NH * TC], f32)
            for kc in range(NH):
                ps = psum_sm.tile([P, TC], f32, tag="sm", name="ps")
                for hc in range(NH):
                    nc.tensor.matmul(
                        ps[:],
                        sb_wqt[:, hc * H + kc * P: hc * H + (kc + 1) * P],
                        sb_decT[:, hc * TC:(hc + 1) * TC],
                        start=(hc == 0), stop=(hc == NH - 1),
                    )
                nc.vector.tensor_scalar_add(
                    sb_wqb[:, kc * TC:(kc + 1) * TC], ps[:],
                    sb_bqb[:, kc:kc + 1])

            # uhT[k, s], chunked over k: sb_uh[:, kc*S + s]
            sb_uh = consts.tile([P, NH * S], f32)
            for kc in range(NH):
                ps = psum_big.tile([P, S], f32, tag="big", name="psb")
                for hc in range(NH):
                    nc.tensor.matmul(
                        ps[:],
                        sb_wct[:, hc * H + kc * P: hc * H + (kc + 1) * P],
                        sb_encT[:, hc * S:(hc + 1) * S],
                        start=(hc == 0), stop=(hc == NH - 1),
                    )
                nc.vector.tensor_copy(sb_uh[:, kc * S:(kc + 1) * S], ps[:])

            # scores[t, s] accumulated in one PSUM tile [64, 512],
            # two independent 32-row stripes so softmax of stripe 0 can
            # start while stripe 1 still accumulates
            if SCORES_MODE == "f32r":
                sb_V = consts.tile([P, NH * 64], tanh_dt)
                nc.vector.tensor_copy(sb_V[:], sb_V0[:])
            else:
                sb_V = sb_V0

            ps_scores = [
                psum_sc.tile([32, S], f32, tag="scores0", name="scores0"),
                psum_sc.tile([32, S], f32, tag="scores1", name="scores1"),
            ]

            # groups are hc-major: 8 t-slots sharing one uh chunk, so the
            # first tanh only needs uh[0], and a phase's adds all read the
            # same uh chunk. GB t's per group, NG groups per (stripe, hc).
            GB = FD // S              # 8 t-slots per ACT batch
            NG = 32 // GB             # 4 groups per (stripe, hc) phase
            N_GP_OFF = 2              # slots per group offloaded to GpSimd

            def do_stripe(half, phases=range(NH)):
                h0 = half * 32
                for hc in phases:
                    for g in range(NG):
                        sum_t = sums.tile([P, FD], f32, tag="sum", name="sum_t")
                        for tt in range(GB):
                            t = h0 + g * GB + tt
                            eng = (nc.gpsimd if tt >= GB - N_GP_OFF
                                   else nc.vector)
                            eng.tensor_scalar_add(
                                sum_t[:, tt * S:(tt + 1) * S],
                                sb_uh[:, hc * S:(hc + 1) * S],
                                sb_wqb[:, hc * TC + t: hc * TC + t + 1])
                        tanh_t = tanhs.tile([P, FD], tanh_dt, tag="tanh",
                                            name="tanh_t")
                        nc.scalar.activation(tanh_t[:], sum_t[:], AF.Tanh)
                        for tt in range(GB):
                            t = h0 + g * GB + tt
                            tl = t - h0
                            lhsT = sb_V[:, hc * 2 * 32 + 32 - tl:
                                        hc * 2 * 32 + 2 * 32 - tl]
                            rhs = tanh_t[:, tt * S:(tt + 1) * S]
                            nc.tensor.matmul(
                                ps_scores[half][:, :], lhsT, rhs,
                                start=(hc == 0 and g == 0 and tt == 0),
                                stop=(hc == NH - 1 and g == NG - 1
                                      and tt == GB - 1),
                                skip_group_check=True,
                            )

            sb_aligns = [consts.tile([32, S], f32, name="sb_align0"),
                         consts.tile([32, S], f32, name="sb_align1")]
            sb_alignT = consts.tile([P, NS * TC], bf16)
            sb_cT = consts.tile([P, NH * TC], bf16)
            sb_attn = consts.tile([P, NH * TC], f32)

            def do_epilogue(half):
                h0 = half * 32
                sb_align = sb_aligns[half]
                psc = ps_scores[half]
                # softmax over s for this 32-row stripe
                mx = small.tile([32, 1], f32, tag="st", name="mx")
                negmax = small.tile([32, 1], f32, tag="st", name="negmax")
                sumexp = small.tile([32, 1], f32, tag="st", name="sumexp")
                rsum = small.tile([32, 1], f32, tag="st", name="rsum")
                nc.vector.reduce_max(mx[:], psc[:, :],
                                     axis=mybir.AxisListType.X)
                nc.vector.tensor_scalar_mul(negmax[:], mx[:], -1.0)
                nc.scalar.activation(sb_align[:, :], psc[:, :], AF.Exp,
                                     bias=negmax[:], accum_out=sumexp[:])
                nc.vector.reciprocal(rsum[:], sumexp[:])
                nc.vector.tensor_scalar_mul(sb_align[:, :], sb_align[:, :],
                                            rsum[:])
                nc.sync.dma_start(d_align[h0:h0 + 32, :], sb_align[:, :])

                # alignT[s, t-half] via PE transpose of [32, 128] blocks
                for sc in range(NS):
                    pst = psum_sm.tile([P, 32], f32, tag="sm", name="pst")
                    nc.tensor.transpose(
                        pst[:], sb_align[:, sc * P:(sc + 1) * P],
                        ident[:, :])
                    nc.vector.tensor_copy(
                        sb_alignT[:, sc * TC + h0: sc * TC + h0 + 32], pst[:])

                # cT[h, t-half] = sum_s enc[s, h] * alignT[s, t]
                for kc in range(NH):
                    ps = psum_sm.tile([P, 32], f32, tag="sm", name="ps")
                    for sc in range(NS):
                        nc.tensor.matmul(
                            ps[:],
                            sb_enc[:, sc * H + kc * P: sc * H + (kc + 1) * P],
                            sb_alignT[:, sc * TC + h0: sc * TC + h0 + 32],
                            start=(sc == 0), stop=(sc == NS - 1),
                        )
                    nc.vector.tensor_copy(
                        sb_cT[:, kc * TC + h0: kc * TC + h0 + 32], ps[:])

                # attn_hT[k, t-half] = sum_d Wo[k, d] * concatT[d, t] + bo[k]
                for kc in range(NH):
                    ps = psum_sm.tile([P, 32], f32, tag="ps2", name="ps2")
                    for dc in range(2 * NH):
                        if dc < NH:
                            rhs = sb_cT[:, dc * TC + h0: dc * TC + h0 + 32]
                        else:
                            rhs = sb_decTb[:, (dc - NH) * TC + h0:
                                           (dc - NH) * TC + h0 + 32]
                        nc.tensor.matmul(
                            ps[:],
                            sb_wot[:, dc * H + kc * P: dc * H + (kc + 1) * P],
                            rhs,
                            start=(dc == 0), stop=(dc == 2 * NH - 1),
                        )
                    nc.vector.tensor_scalar_add(
                        sb_attn[:, kc * TC + h0: kc * TC + h0 + 32], ps[:],
                        sb_bob[:, kc:kc + 1])
                nc.sync.dma_start(
                    d_attn[:].rearrange("p (k t) -> p k t", k=NH)[:, :, h0:h0 + 32],
                    sb_attn[:].rearrange("p (k t) -> p k t", k=NH)[:, :, h0:h0 + 32])

            do_stripe(0)
            do_stripe(1, phases=[0])
            do_epilogue(0)
            do_stripe(1, phases=[1, 2, 3])
            do_epilogue(1)

    nc.compile()
    return nc


def _get_nc():
    global _cached
    if _cached is None:
        _cached = _build()
    return _cached


def _chunk_cols(a):
    """[n*128, C] -> [128, n*C] with row-chunk i at cols [i*C:(i+1)*C]."""
    n = a.shape[0] // P
    return np.ascontiguousarray(
        a.reshape(n, P, a.shape[1]).transpose(1, 0, 2).reshape(P, -1))


def make_in_maps(dec_output, enc_output, Wq, bq, Wc, v, Wo, bo):
    if SCORES_MODE == "bf16":
        import ml_dtypes
        v_dt = ml_dtypes.bfloat16
    else:
        v_dt = np.float32

    import ml_dtypes
    bf16 = ml_dtypes.bfloat16
    wqt = _chunk_cols(np.ascontiguousarray(Wq.T))
    wct = _chunk_cols(np.ascontiguousarray(Wc.T))
    wot = _chunk_cols(np.ascontiguousarray(Wo.T)).astype(bf16)
    bqb = np.ascontiguousarray(bq.reshape(NH, P).T)
    bob = np.ascontiguousarray(bo.reshape(NH, P).T)
    # Sliding-window padded v: column (hc*64 + 32) holds v chunk hc; the
    # lhsT slice [hc*64 + 32 - tl : hc*64 + 64 - tl] puts v at window col tl
    V = np.zeros((P, NH, 64), dtype=np.float32)
    for hc in range(NH):
        V[:, hc, 32] = v[hc * P:(hc + 1) * P]
    V = np.ascontiguousarray(V.reshape(P, NH * 64).astype(v_dt))

    in_maps = []
    for c in range(N_CORES):
        b, th = c // 2, c % 2
        t0 = th * TC
        decT = _chunk_cols(np.ascontiguousarray(dec_output[b].T[:, t0:t0 + TC]))
        enc_sb = _chunk_cols(enc_output[b]).astype(bf16)
        encT_sb = _chunk_cols(np.ascontiguousarray(enc_output[b].T))
        in_maps.append({
            "decT": decT, "decTb": decT.astype(bf16), "enc": enc_sb,
            "encT": encT_sb, "wqt": wqt, "wct": wct, "wot": wot,
            "V": V, "bqb": bqb, "bob": bob,
        })
    return in_maps


def assemble(results):
    attn_h = np.empty((B, T, H), dtype=np.float32)
    align_vectors = np.empty((T, B, S), dtype=np.float32)
    for c in range(N_CORES):
        b, th = c // 2, c % 2
        t0 = th * TC
        align_vectors[t0:t0 + TC, b, :] = results[c]["align_out"]
        a = results[c]["attn_out"].reshape(P, NH, TC)
        attn_h[b, t0:t0 + TC, :] = a.transpose(2, 1, 0).reshape(TC, H)
    return attn_h, align_vectors


def run(trace=False, **inputs):
    from concourse.bass_utils import run_bass_kernel_spmd

    args = {k: np.asarray(inputs[k], dtype=np.float32)
            for k in ("dec_output", "enc_output", "Wq", "bq", "Wc", "v",
                      "Wo", "bo")}
    nc = _get_nc()
    in_maps = make_in_maps(**args)
    if trace:
        try:
            from antenv.axon_hooks import set_axon_ntff_profile_hook
            from trn_agent_boot.trn_boot import _ntff_profile_via_ctypes
            set_axon_ntff_profile_hook(
                _ntff_profile_via_ctypes("/opt/axon/libaxon_pjrt.so"))
        except Exception:
            pass
    res = run_bass_kernel_spmd(nc, in_maps, core_ids=list(range(N_CORES)),
                               trace=trace)
    out = assemble(res.results)
    return out, res


def kernel(**inputs):
    out, _ = run(trace=False, **inputs)
    return out


# revision 9
# speedup vs baseline: 3.6439x; 3.6439x over previous
"""Trainium2 Bass kernel for Bahdanau (MLP) additive attention.

Reference computation (B=4, T=128, S=512, H=512):
    wq = dec @ Wq.T + bq                    [B,T,H]
    uh = enc @ Wc.T                         [B,S,H]
    scores[b,t,s] = sum_h v[h] * tanh(wq[b,t,h] + uh[b,s,h])
    align = softmax(scores, axis=-1)        [B,T,S]
    c = align @ enc                         [B,T,H]
    attn_h = [c, dec] @ Wo.T + bo           [B,T,H]
    returns (attn_h, align.transpose(1,0,2))

Sharding: pure data parallel over (batch, T-half) -> 8 cores, 64 queries
per core, no cross-core communication.

Per-core dataflow (all layouts keep the hidden index on partitions):
    PE:  wqT = Wq.T-chunks @ decT   (+bq via DVE)         [h, t]
    PE:  uhT = Wc.T-chunks @ encT                          [h, s]
    DVE: sum(t,hc) = uhT[hc] + wqT[hc, t]  (per-partition scalar add)
    ACT: tanh over big batched tiles [128, 4096]
    PE:  scores[0:64, :] += V_window(t, hc).T @ tanh(t,hc)  (padded-v
         stationary trick: column t of the 64-wide window holds v_chunk,
         all other columns zero, so one M=64 matmul accumulates row t and
         adds zero elsewhere)
    DVE/ACT: softmax rows (reduce_max, exp with -max bias + accum row sum,
         reciprocal, scale)
    PE:  transpose align -> alignT; cT = enc-chunks @ alignT;
         attn_hT = Wo.T-chunks @ [cT; decT]  (+bo via DVE)
"""

import numpy as np

B, T, S, H = 4, 128, 512, 512
P = 128
NH = H // P          # 4 h-chunks
NS = S // P          # 4 s-chunks
TC = 64              # queries per core
TB = 2               # queries per ACT batch
FD = TB * NH * S     # 4096 free dim of the batched tanh tile
N_CORES = 8

# "f32r" (full-rate fp32-reduced matmul), "bf16", or "f32" (4x slower PE)
SCORES_MODE = "f32r"

_cached = None


def _build():
    import concourse.bacc as bacc
    import concourse.tile as tile
    import concourse.mybir as mybir
    from concourse.masks import make_identity

    f32 = mybir.dt.float32
    AF = mybir.ActivationFunctionType

    if SCORES_MODE == "bf16":
        tanh_dt = mybir.dt.bfloat16
    elif SCORES_MODE == "f32r":
        tanh_dt = mybir.dt.float32r
    else:
        tanh_dt = f32

    nc = bacc.Bacc("TRN2", target_bir_lowering=False, debug=False,
                   num_devices=N_CORES)

    d_decT = nc.dram_tensor("decT", [P, NH * TC], f32, kind="ExternalInput")
    bf16 = mybir.dt.bfloat16
    d_enc = nc.dram_tensor("enc", [P, NS * H], bf16, kind="ExternalInput")
    d_decTb = nc.dram_tensor("decTb", [P, NH * TC], bf16, kind="ExternalInput")
    uh_in_dt = mybir.dt.float32r if SCORES_MODE == "f32r" else f32
    d_encT = nc.dram_tensor("encT", [P, NH * S], uh_in_dt, kind="ExternalInput")
    d_wqt = nc.dram_tensor("wqt", [P, NH * H], f32, kind="ExternalInput")
    d_wct = nc.dram_tensor("wct", [P, NH * H], uh_in_dt, kind="ExternalInput")
    d_wot = nc.dram_tensor("wot", [P, 2 * NH * H], bf16, kind="ExternalInput")
    d_V_dt = mybir.dt.bfloat16 if SCORES_MODE == "bf16" else f32
    d_V = nc.dram_tensor("V", [P, NH * 64], d_V_dt, kind="ExternalInput")
    d_bqb = nc.dram_tensor("bqb", [P, NH], f32, kind="ExternalInput")
    d_bob = nc.dram_tensor("bob", [P, NH], f32, kind="ExternalInput")

    d_align = nc.dram_tensor("align_out", [TC, S], f32, kind="ExternalOutput")
    d_attn = nc.dram_tensor("attn_out", [P, NH * TC], f32, kind="ExternalOutput")


    with tile.TileContext(nc) as tc:
        with (
            tc.tile_pool(name="consts", bufs=1) as consts,
            tc.tile_pool(name="sums", bufs=4) as sums,
            tc.tile_pool(name="tanhs", bufs=3) as tanhs,
            tc.tile_pool(name="small", bufs=4) as small,
            tc.tile_pool(name="psc", bufs=1, space="PSUM") as psum_sc,
            tc.tile_pool(name="pbig", bufs=2, space="PSUM") as psum_big,
            tc.tile_pool(name="psm", bufs=2, space="PSUM") as psum_sm,
        ):
            uh_dt = mybir.dt.float32r if SCORES_MODE == "f32r" else f32
            sb_decT = consts.tile([P, NH * TC], f32)
            nc.sync.dma_start(sb_decT[:], d_decT[:])
            sb_wqt = consts.tile([P, NH * H], f32)
            nc.sync.dma_start(sb_wqt[:], d_wqt[:])
            sb_wct = consts.tile([P, NH * H], uh_dt)
            nc.sync.dma_start(sb_wct[:], d_wct[:])
            sb_encT = consts.tile([P, NH * S], uh_dt)
            nc.sync.dma_start(sb_encT[:], d_encT[:])
            sb_V0 = consts.tile([P, NH * 64], d_V_dt)
            nc.sync.dma_start(sb_V0[:], d_V[:])
            sb_bqb = consts.tile([P, NH], f32)
            nc.sync.dma_start(sb_bqb[:], d_bqb[:])
            sb_bob = consts.tile([P, NH], f32)
            nc.sync.dma_start(sb_bob[:], d_bob[:])
            sb_enc = consts.tile([P, NS * H], bf16)
            nc.sync.dma_start(sb_enc[:], d_enc[:])
            sb_decTb = consts.tile([P, NH * TC], bf16)
            nc.sync.dma_start(sb_decTb[:], d_decTb[:])
            sb_wot = consts.tile([P, 2 * NH * H], bf16)
            nc.sync.dma_start(sb_wot[:], d_wot[:])
            ident = consts.tile([32, 32], f32)
            make_identity(nc, ident[:, :])

            # wqT[k, t] (+bq), chunked over k: sb_wqb[:, kc*TC + t]
            sb_wqb = consts.tile([P, NH * TC], f32)
            for kc in range(NH):
                ps = psum_sm.tile([P, TC], f32, tag="sm", name="ps")
                for hc in range(NH):
                    nc.tensor.matmul(
                        ps[:],
                        sb_wqt[:, hc * H + kc * P: hc * H + (kc + 1) * P],
                        sb_decT[:, hc * TC:(hc + 1) * TC],
                        start=(hc == 0), stop=(hc == NH - 1),
                    )
                nc.vector.tensor_scalar_add(
                    sb_wqb[:, kc * TC:(kc + 1) * TC], ps[:],
                    sb_bqb[:, kc:kc + 1])

            # uhT[k, s], chunked over k: sb_uh[:, kc*S + s]
            sb_uh = consts.tile([P, NH * S], f32)
            for kc in range(NH):
                ps = psum_big.tile([P, S], f32, tag="big", name="psb")
                for hc in range(NH):
                    nc.tensor.matmul(
                        ps[:],
                        sb_wct[:, hc * H + kc * P: hc * H + (kc + 1) * P],
                        sb_encT[:, hc * S:(hc + 1) * S],
                        start=(hc == 0), stop=(hc == NH - 1),
                    )
                nc.vector.tensor_copy(sb_uh[:, kc * S:(kc + 1) * S], ps[:])

            # scores[t, s] accumulated in one PSUM tile [64, 512],
            # two independent 32-row stripes so softmax of stripe 0 can
            # start while stripe 1 still accumulates
            if SCORES_MODE == "f32r":
                sb_V = consts.tile([P, NH * 64], tanh_dt)
                nc.vector.tensor_copy(sb_V[:], sb_V0[:])
            else:
                sb_V = sb_V0

            ps_scores = [
                psum_sc.tile([32, S], f32, tag="scores0", name="scores0"),
                psum_sc.tile([32, S], f32, tag="scores1", name="scores1"),
            ]

            # groups are hc-major: 8 t-slots sharing one uh chunk, so the
            # first tanh only needs uh[0], and a phase's adds all read the
            # same uh chunk. GB t's per group, NG groups per (stripe, hc).
            GB = FD // S              # 8 t-slots per ACT batch
            NG = 32 // GB             # 4 groups per (stripe, hc) phase
            N_GP_OFF = 0              # slots per group offloaded to GpSimd

            def do_stripe(half, phases=range(NH)):
                h0 = half * 32
                for hc in phases:
                    for g in range(NG):
                        sum_t = sums.tile([P, FD], f32, tag="sum", name="sum_t")
                        for tt in range(GB):
                            t = h0 + g * GB + tt
                            eng = (nc.gpsimd if tt >= GB - N_GP_OFF
                                   else nc.vector)
                            eng.tensor_scalar_add(
                                sum_t[:, tt * S:(tt + 1) * S],
                                sb_uh[:, hc * S:(hc + 1) * S],
                                sb_wqb[:, hc * TC + t: hc * TC + t + 1])
                        tanh_t = tanhs.tile([P, FD], tanh_dt, tag="tanh",
                                            name="tanh_t")
                        nc.scalar.activation(tanh_t[:], sum_t[:], AF.Tanh)
                        for tt in range(GB):
                            t = h0 + g * GB + tt
                            tl = t - h0
                            lhsT = sb_V[:, hc * 2 * 32 + 32 - tl:
                                        hc * 2 * 32 + 2 * 32 - tl]
                            rhs = tanh_t[:, tt * S:(tt + 1) * S]
                            nc.tensor.matmul(
                                ps_scores[half][:, :], lhsT, rhs,
                                start=(hc == 0 and g == 0 and tt == 0),
                                stop=(hc == NH - 1 and g == NG - 1
                                      and tt == GB - 1),
                                skip_group_check=True,
                            )

            sb_aligns = [consts.tile([32, S], f32, name="sb_align0"),
                         consts.tile([32, S], f32, name="sb_align1")]
            sb_alignT = consts.tile([P, NS * TC], bf16)
            sb_cT = consts.tile([P, NH * TC], bf16)
            sb_attn = consts.tile([P, NH * TC], f32)

            def do_epilogue(half):
                h0 = half * 32
                sb_align = sb_aligns[half]
                psc = ps_scores[half]
                # softmax over s for this 32-row stripe
                mx = small.tile([32, 1], f32, tag="st", name="mx")
                negmax = small.tile([32, 1], f32, tag="st", name="negmax")
                sumexp = small.tile([32, 1], f32, tag="st", name="sumexp")
                rsum = small.tile([32, 1], f32, tag="st", name="rsum")
                nc.vector.reduce_max(mx[:], psc[:, :],
                                     axis=mybir.AxisListType.X)
                nc.vector.tensor_scalar_mul(negmax[:], mx[:], -1.0)
                nc.scalar.activation(sb_align[:, :], psc[:, :], AF.Exp,
                                     bias=negmax[:], accum_out=sumexp[:])
                nc.vector.reciprocal(rsum[:], sumexp[:])
                nc.vector.tensor_scalar_mul(sb_align[:, :], sb_align[:, :],
                                            rsum[:])
                nc.sync.dma_start(d_align[h0:h0 + 32, :], sb_align[:, :])

                # alignT[s, t-half] via PE transpose of [32, 128] blocks
                for sc in range(NS):
                    pst = psum_sm.tile([P, 32], f32, tag="sm", name="pst")
                    nc.tensor.transpose(
                        pst[:], sb_align[:, sc * P:(sc + 1) * P],
                        ident[:, :])
                    nc.vector.tensor_copy(
                        sb_alignT[:, sc * TC + h0: sc * TC + h0 + 32], pst[:])

                # cT[h, t-half] = sum_s enc[s, h] * alignT[s, t]
                for kc in range(NH):
                    ps = psum_sm.tile([P, 32], f32, tag="sm", name="ps")
                    for sc in range(NS):
                        nc.tensor.matmul(
                            ps[:],
                            sb_enc[:, sc * H + kc * P: sc * H + (kc + 1) * P],
                            sb_alignT[:, sc * TC + h0: sc * TC + h0 + 32],
                            start=(sc == 0), stop=(sc == NS - 1),
                        )
                    nc.vector.tensor_copy(
                        sb_cT[:, kc * TC + h0: kc * TC + h0 + 32], ps[:])

                # attn_hT[k, t-half] = sum_d Wo[k, d] * concatT[d, t] + bo[k]
                for kc in range(NH):
                    ps = psum_sm.tile([P, 32], f32, tag="ps2", name="ps2")
                    for dc in range(2 * NH):
                        if dc < NH:
                            rhs = sb_cT[:, dc * TC + h0: dc * TC + h0 + 32]
                        else:
                            rhs = sb_decTb[:, (dc - NH) * TC + h0:
                                           (dc - NH) * TC + h0 + 32]
                        nc.tensor.matmul(
                            ps[:],
                            sb_wot[:, dc * H + kc * P: dc * H + (kc + 1) * P],
                            rhs,
                            start=(dc == 0), stop=(dc == 2 * NH - 1),
                        )
                    nc.vector.tensor_scalar_add(
                        sb_attn[:, kc * TC + h0: kc * TC + h0 + 32], ps[:],
                        sb_bob[:, kc:kc + 1])
                nc.sync.dma_start(
                    d_attn[:].rearrange("p (k t) -> p k t", k=NH)[:, :, h0:h0 + 32],
                    sb_attn[:].rearrange("p (k t) -> p k t", k=NH)[:, :, h0:h0 + 32])

            do_stripe(0)
            do_stripe(1, phases=[0])
            do_epilogue(0)
            do_stripe(1, phases=[1, 2, 3])
            do_epilogue(1)

    nc.compile()
    return nc


def _get_nc():
    global _cached
    if _cached is None:
        _cached = _build()
    return _cached


def _chunk_cols(a):
    """[n*128, C] -> [128, n*C] with row-chunk i at cols [i*C:(i+1)*C]."""
    n = a.shape[0] // P
    return np.ascontiguousarray(
        a.reshape(n, P, a.shape[1]).transpose(1, 0, 2).reshape(P, -1))


def make_in_maps(dec_output, enc_output, Wq, bq, Wc, v, Wo, bo):
    if SCORES_MODE == "bf16":
        import ml_dtypes
        v_dt = ml_dtypes.bfloat16
    else:
        v_dt = np.float32

    import ml_dtypes
    bf16 = ml_dtypes.bfloat16
    wqt = _chunk_cols(np.ascontiguousarray(Wq.T))
    wct = _chunk_cols(np.ascontiguousarray(Wc.T))
    wot = _chunk_cols(np.ascontiguousarray(Wo.T)).astype(bf16)
    bqb = np.ascontiguousarray(bq.reshape(NH, P).T)
    bob = np.ascontiguousarray(bo.reshape(NH, P).T)
    # Sliding-window padded v: column (hc*64 + 32) holds v chunk hc; the
    # lhsT slice [hc*64 + 32 - tl : hc*64 + 64 - tl] puts v at window col tl
    V = np.zeros((P, NH, 64), dtype=np.float32)
    for hc in range(NH):
        V[:, hc, 32] = v[hc * P:(hc + 1) * P]
    V = np.ascontiguousarray(V.reshape(P, NH * 64).astype(v_dt))

    in_maps = []
    for c in range(N_CORES):
        b, th = c // 2, c % 2
        t0 = th * TC
        decT = _chunk_cols(np.ascontiguousarray(dec_output[b].T[:, t0:t0 + TC]))
        enc_sb = _chunk_cols(enc_output[b]).astype(bf16)
        encT_sb = _chunk_cols(np.ascontiguousarray(enc_output[b].T))
        in_maps.append({
            "decT": decT, "decTb": decT.astype(bf16), "enc": enc_sb,
            "encT": encT_sb, "wqt": wqt, "wct": wct, "wot": wot,
            "V": V, "bqb": bqb, "bob": bob,
        })
    return in_maps


def assemble(results):
    attn_h = np.empty((B, T, H), dtype=np.float32)
    align_vectors = np.empty((T, B, S), dtype=np.float32)
    for c in range(N_CORES):
        b, th = c // 2, c % 2
        t0 = th * TC
        align_vectors[t0:t0 + TC, b, :] = results[c]["align_out"]
        a = results[c]["attn_out"].reshape(P, NH, TC)
        attn_h[b, t0:t0 + TC, :] = a.transpose(2, 1, 0).reshape(TC, H)
    return attn_h, align_vectors


def run(trace=False, **inputs):
    from concourse.bass_utils import run_bass_kernel_spmd

    args = {k: np.asarray(inputs[k], dtype=np.float32)
            for k in ("dec_output", "enc_output", "Wq", "bq", "Wc", "v",
                      "Wo", "bo")}
    nc = _get_nc()
    in_maps = make_in_maps(**args)
    if trace:
        try:
            from antenv.axon_hooks import set_axon_ntff_profile_hook
            from trn_agent_boot.trn_boot import _ntff_profile_via_ctypes
            set_axon_ntff_profile_hook(
                _ntff_profile_via_ctypes("/opt/axon/libaxon_pjrt.so"))
        except Exception:
            pass
    res = run_bass_kernel_spmd(nc, in_maps, core_ids=list(range(N_CORES)),
                               trace=trace)
    out = assemble(res.results)
    return out, res


def kernel(**inputs):
    out, _ = run(trace=False, **inputs)
    return out
